# revision 1
# baseline (speedup 1.0000x reference)
import numpy as np

B = 128
FEAT = 64
LATENT = 512
OUT_F = 6144  # NUM_POINTS * 3
EPS = 1e-5
N_CORES = 8
SEGS_PER_CORE = 16
S_PAD = 8192
FMIN = np.float32(np.finfo(np.float32).min)

_CACHE = {}


def build_nc():
    from concourse import bass, bacc, tile

    mybir = bass.mybir
    f32 = mybir.dt.float32
    f32r = mybir.dt.float32r
    AF = mybir.ActivationFunctionType

    nc = bacc.Bacc("TRN2")
    xt_d = nc.declare_dram_parameter("xt", [128, 8, S_PAD], f32, isOutput=False)
    wp_d = nc.declare_dram_parameter("wp2", [128, LATENT], f32, isOutput=False)
    bp_d = nc.declare_dram_parameter("bp", [128, 4], f32, isOutput=False)
    w1_d = nc.declare_dram_parameter("w1p", [128, 1024], f32, isOutput=False)
    b1_d = nc.declare_dram_parameter("b1p", [128, 2], f32, isOutput=False)
    w2_d = nc.declare_dram_parameter("w2p", [128, 1024], f32, isOutput=False)
    b2_d = nc.declare_dram_parameter("b2p", [128, 4], f32, isOutput=False)
    w3_d = nc.declare_dram_parameter("w3p", [128, 3, 4, 2048], f32r, isOutput=False)
    selT_d = nc.declare_dram_parameter("selT", [128, 2], f32, isOutput=False)
    sel_d = nc.declare_dram_parameter("sel", [2, 128], f32, isOutput=False)
    out_d = nc.declare_dram_parameter("out", [16, OUT_F], f32, isOutput=True)

    with tile.TileContext(nc) as tc:
        with (
            tc.tile_pool(name="wpool", bufs=1) as wpool,
            tc.tile_pool(name="fpool", bufs=4) as fpool,
            tc.tile_pool(name="spool", bufs=1) as spool,
            tc.tile_pool(name="ps_s", bufs=1, space=bass.MemorySpace.PSUM) as ps_s,
            tc.tile_pool(name="ps_b", bufs=1, space=bass.MemorySpace.PSUM) as ps_b,
            tc.tile_pool(name="ps_m", bufs=3, space=bass.MemorySpace.PSUM) as ps_m,
            tc.tile_pool(name="ps_o", bufs=3, space=bass.MemorySpace.PSUM) as ps_o,
        ):
            wp = wpool.tile([128, LATENT], f32)
            bp = wpool.tile([128, 4], f32)
            w1 = wpool.tile([128, 1024], f32)
            b1 = wpool.tile([128, 2], f32)
            w2 = wpool.tile([128, 1024], f32)
            b2 = wpool.tile([128, 4], f32)
            # small weights on the gpsimd SWDGE queue: keeps both HW queues
            # free for feat streaming from t=0
            for t, d in (
                (wp, wp_d), (bp, bp_d), (w1, w1_d), (b1, b1_d),
                (w2, w2_d), (b2, b2_d),
            ):
                nc.gpsimd.dma_start(t[:], d[:])

            # group-selector matrices for the two-group LayerNorm
            sel2T = wpool.tile([128, 2], f32)  # sel2T[p, j] = (p // 64 == j)
            sel2 = wpool.tile([2, 128], f32)  # sel2[j, p] = (p // 64 == j)
            eps_t = wpool.tile([2, 1], f32)
            nc.gpsimd.dma_start(sel2T[:], selT_d[:])
            nc.gpsimd.dma_start(sel2[:], sel_d[:])
            nc.vector.memset(eps_t[:], EPS)

            val = spool.tile([128, 8], f32)
            valw = spool.tile([128, 4], f32)
            zc = spool.tile([128, 8], f32)
            zsq = spool.tile([128, 8], f32)
            zn16 = spool.tile([128, 16], f32)
            mu2 = spool.tile([2, 8], f32)
            std = spool.tile([2, 8], f32)
            rstd = spool.tile([2, 8], f32)
            lat = spool.tile([128, 64], f32)
            h1 = spool.tile([128, 32], f32)
            h2 = spool.tile([128, 64], f32r)
            out_sb = spool.tile([16, OUT_F], f32)
            nc.vector.memset(zn16[:], 0.0)

            # --- segment max pooling: even tiles on SP queue, odd on Act.
            # last tile per queue split into half-DMAs to shorten the tail ---
            H = S_PAD // 2
            for t in range(6):
                ft = fpool.tile([128, S_PAD], f32, name="ft")
                eng = nc.sync if t % 2 == 0 else nc.scalar
                eng.dma_start(ft[:], xt_d[:, t, :])
                nc.vector.reduce_max(
                    val[:, t : t + 1], ft[:], axis=mybir.AxisListType.X
                )
            for h in range(2):
                for t in (6, 7):
                    fh = fpool.tile([128, H], f32, name="ft")
                    eng = nc.sync if t % 2 == 0 else nc.scalar
                    eng.dma_start(fh[:], xt_d[:, t, H * h : H * (h + 1)])
                    nc.vector.reduce_max(
                        valw[:, 2 * h + t - 6 : 2 * h + t - 5], fh[:],
                        axis=mybir.AxisListType.X,
                    )
            nc.vector.tensor_tensor(
                val[:, 6:8], valw[:, 0:2], valw[:, 2:4], op=mybir.AluOpType.max
            )

            # w3 tiles share the feat ring (name "ft"): the WAR dep on each
            # aliased feat tile's reduce keeps these DMAs behind the stream
            w3t = [fpool.tile([128, 4, 2048], f32r, name="ft") for i in range(3)]
            for g in range(3):
                nc.sync.dma_start(w3t[g][:], w3_d[:, g, :, :])

            # --- LayerNorm per (group, col) directly on val [128, 8] ---
            red = ps_s.tile([2, 8], f32, name="red")
            nc.tensor.matmul(red[:], sel2T[:], val[:], start=True, stop=True)
            nc.scalar.mul(mu2[:], red[:], 1.0 / FEAT)
            bc = ps_b.tile([128, 8], f32, name="bc")
            nc.tensor.matmul(bc[:], sel2[:], mu2[:], start=True, stop=True)
            nc.vector.tensor_tensor(zc[:], val[:], bc[:], op=mybir.AluOpType.subtract)
            nc.scalar.activation(zsq[:], zc[:], AF.Square)
            red2 = ps_s.tile([2, 8], f32, name="red")
            nc.tensor.matmul(red2[:], sel2T[:], zsq[:], start=True, stop=True)
            nc.scalar.activation(std[:], red2[:], AF.Sqrt, bias=eps_t[:], scale=1.0 / FEAT)
            nc.vector.reciprocal(rstd[:], std[:])
            bc2 = ps_b.tile([128, 8], f32, name="bc")
            nc.tensor.matmul(bc2[:], sel2[:], rstd[:], start=True, stop=True)
            # scatter normalized groups into disjoint columns of zn16
            nc.vector.tensor_tensor(
                zn16[0:64, 0:8], zc[0:64, :], bc2[0:64, :], op=mybir.AluOpType.mult
            )
            nc.vector.tensor_tensor(
                zn16[64:128, 8:16], zc[64:128, :], bc2[64:128, :],
                op=mybir.AluOpType.mult,
            )

            # --- proj (ln affine folded into wp/bp): lat[128m+p, s] ---
            for m in range(4):
                ps = ps_m.tile([128, 16], f32)
                nc.tensor.matmul(
                    ps[:], wp[:, 128 * m : 128 * (m + 1)], zn16[:],
                    start=True, stop=True,
                )
                nc.scalar.activation(
                    lat[:, 16 * m : 16 * (m + 1)], ps[:], AF.Identity,
                    bias=bp[:, m : m + 1],
                )

            # --- h1 = relu(latent @ w1 + b1), transposed ---
            for n in range(2):
                ps = ps_m.tile([128, 16], f32)
                for k in range(4):
                    nc.tensor.matmul(
                        ps[:],
                        w1[:, (k * 2 + n) * 128 : (k * 2 + n + 1) * 128],
                        lat[:, 16 * k : 16 * (k + 1)],
                        start=(k == 0), stop=(k == 3),
                    )
                nc.scalar.activation(
                    h1[:, 16 * n : 16 * (n + 1)], ps[:], AF.Relu,
                    bias=b1[:, n : n + 1],
                )

            # --- h2 = relu(h1 @ w2 + b2), transposed ---
            for n in range(4):
                ps = ps_m.tile([128, 16], f32)
                for k in range(2):
                    nc.tensor.matmul(
                        ps[:],
                        w2[:, (k * 4 + n) * 128 : (k * 4 + n + 1) * 128],
                        h1[:, 16 * k : 16 * (k + 1)],
                        start=(k == 0), stop=(k == 1),
                    )
                nc.scalar.activation(
                    h2[:, 16 * n : 16 * (n + 1)], ps[:], AF.Relu,
                    bias=b2[:, n : n + 1],
                )

            # --- out[s, n] = h2.T @ w3: h2 as stationary lhsT (fp32r) ---
            for g in range(3):
                for sub in range(4):
                    n0 = 2048 * g + 512 * sub
                    ps = ps_o.tile([16, 512], f32, name="pso")
                    for k in range(4):
                        nc.tensor.matmul(
                            ps[:],
                            h2[:, 16 * k : 16 * (k + 1)],
                            w3t[g][:, k, 512 * sub : 512 * (sub + 1)],
                            start=(k == 0), stop=(k == 3),
                        )
                    nc.vector.tensor_copy(out_sb[:, n0 : n0 + 512], ps[:])
                nc.sync.dma_start(
                    out_d[:, 2048 * g : 2048 * (g + 1)],
                    out_sb[:, 2048 * g : 2048 * (g + 1)],
                )

    nc.finalize()
    return nc


def round_f32r(x):
    # fp32r = fp32 RNE-rounded to 11 mantissa bits (low 12 bits zero)
    u = np.ascontiguousarray(x, np.float32).view(np.uint32)
    lsb = (u >> 12) & 1
    return (((u + 0x7FF + lsb) & 0xFFFFF000).astype(np.uint32)).view(np.float32)


def pack_weights(ln_g, ln_b, proj_w, proj_b, w1, b1, w2, b2, w3, b3):
    c = np.ascontiguousarray
    wp = (ln_g[:, None] * proj_w).astype(np.float32)  # [64, 512]
    bpv = (ln_b.astype(np.float64) @ proj_w.astype(np.float64)).astype(np.float32) + proj_b
    return {
        "wp2": c(np.vstack([wp, wp])),
        "bp": c(bpv.reshape(4, 128).T),
        "w1p": c(w1.reshape(4, 128, 2, 128).transpose(1, 0, 2, 3).reshape(128, 1024)),
        "b1p": c(b1.reshape(2, 128).T),
        "w2p": c(w2.reshape(2, 128, 4, 128).transpose(1, 0, 2, 3).reshape(128, 1024)),
        "b2p": c(b2.reshape(4, 128).T),
        "w3p": round_f32r(w3.reshape(4, 128, 3, 2048).transpose(1, 2, 0, 3)),
        "selT": c(np.repeat(np.eye(2, dtype=np.float32), 64, axis=0)),
        "sel": c(np.repeat(np.eye(2, dtype=np.float32), 64, axis=1)),
    }


def pack_feat_core(feat, bounds, c):
    xt = np.full((128, 8, S_PAD), FMIN, np.float32)
    for sl in range(SEGS_PER_CORE):
        seg = c * SEGS_PER_CORE + sl
        a, b = bounds[seg], bounds[seg + 1]
        blk = feat[a:b]
        L = b - a
        if L > S_PAD:
            blk = np.concatenate(
                [blk[: S_PAD - 1], blk[S_PAD - 1 :].max(0, keepdims=True)], 0
            )
            L = S_PAD
        g, t = divmod(sl, 8)
        if L > 0:
            xt[g * 64 : (g + 1) * 64, t, :L] = blk.T
    return xt


def kernel(**inputs):
    from concourse.bass_utils import run_bass_kernel_spmd

    feat = np.ascontiguousarray(np.asarray(inputs["feat"], dtype=np.float32))
    batch = np.asarray(inputs["batch"])
    wdict = pack_weights(
        *(np.asarray(inputs[k], dtype=np.float32) for k in
          ("ln_g", "ln_b", "proj_w", "proj_b", "w1", "b1", "w2", "b2", "w3", "b3"))
    )

    if "nc" not in _CACHE:
        _CACHE["nc"] = build_nc()
    nc = _CACHE["nc"]

    bounds = np.searchsorted(batch, np.arange(B + 1))
    in_maps = [
        {"xt": pack_feat_core(feat, bounds, c), **wdict} for c in range(N_CORES)
    ]
    res = run_bass_kernel_spmd(nc, in_maps, list(range(N_CORES)))

    out = np.empty((B, OUT_F), np.float32)
    for c in range(N_CORES):
        out[c * 16 : (c + 1) * 16] = res.results[c]["out"]
    out += np.asarray(inputs["b3"], dtype=np.float32)[None, :]
    return out.reshape(B, 2048, 3)



# revision 7
# speedup vs baseline: 1.5974x; 1.5974x over previous
import numpy as np

B = 128
FEAT = 64
LATENT = 512
OUT_F = 6144  # NUM_POINTS * 3
EPS = 1e-5
N_CORES = 8
N_SLOTS = 8  # segment pairs per core
FMIN16 = np.float16(-65504.0)

_CACHE = {}


def build_nc(Ws):
    from concourse import bass, bacc, tile

    mybir = bass.mybir
    f32 = mybir.dt.float32
    f16 = mybir.dt.float16
    AF = mybir.ActivationFunctionType

    W0 = Ws[0]
    nc = bacc.Bacc("TRN2")
    xs_d = [
        nc.declare_dram_parameter(f"x{s}", [128, Ws[s]], f16, isOutput=False)
        for s in range(N_SLOTS)
    ]
    wp_d = nc.declare_dram_parameter("wp2", [128, LATENT], f16, isOutput=False)
    bp_d = nc.declare_dram_parameter("bp", [128, 4], f32, isOutput=False)
    w1_d = nc.declare_dram_parameter("w1p", [128, 1024], f16, isOutput=False)
    b1_d = nc.declare_dram_parameter("b1p", [128, 2], f32, isOutput=False)
    w2_d = nc.declare_dram_parameter("w2p", [128, 1024], f16, isOutput=False)
    b2_d = nc.declare_dram_parameter("b2p", [128, 4], f32, isOutput=False)
    w3_d = nc.declare_dram_parameter("w3n", [128, 12, 4, 512], f16, isOutput=False)
    selT_d = nc.declare_dram_parameter("selT", [128, 2], f32, isOutput=False)
    sel_d = nc.declare_dram_parameter("sel", [2, 128], f32, isOutput=False)
    out_d = nc.declare_dram_parameter("out", [16, OUT_F], f32, isOutput=True)

    with tile.TileContext(nc) as tc:
        with (
            tc.tile_pool(name="wpool", bufs=1) as wpool,
            tc.tile_pool(name="fpool", bufs=3) as fpool,
            tc.tile_pool(name="bpool", bufs=2) as bpool,
            tc.tile_pool(name="w3pool", bufs=1) as w3pool,
            tc.tile_pool(name="spool", bufs=1) as spool,
            tc.tile_pool(name="ps_s", bufs=1, space=bass.MemorySpace.PSUM) as ps_s,
            tc.tile_pool(name="ps_b", bufs=1, space=bass.MemorySpace.PSUM) as ps_b,
            tc.tile_pool(name="ps_m", bufs=3, space=bass.MemorySpace.PSUM) as ps_m,
            tc.tile_pool(name="ps_o", bufs=3, space=bass.MemorySpace.PSUM) as ps_o,
        ):
            wp = wpool.tile([128, LATENT], f16)
            bp = wpool.tile([128, 4], f32)
            w1 = wpool.tile([128, 1024], f16)
            b1 = wpool.tile([128, 2], f32)
            w2 = wpool.tile([128, 1024], f16)
            b2 = wpool.tile([128, 4], f32)
            # small weights on the gpsimd SWDGE queue: keeps both HW queues
            # free for feat streaming from t=0
            for t, d in (
                (wp, wp_d), (bp, bp_d), (w1, w1_d), (b1, b1_d),
                (w2, w2_d), (b2, b2_d),
            ):
                nc.gpsimd.dma_start(t[:], d[:])

            # group-selector matrices for the two-group LayerNorm
            sel2T = wpool.tile([128, 2], f32)  # sel2T[p, j] = (p // 64 == j)
            sel2 = wpool.tile([2, 128], f32)  # sel2[j, p] = (p // 64 == j)
            eps_t = wpool.tile([2, 1], f32)
            nc.gpsimd.dma_start(sel2T[:], selT_d[:])
            nc.gpsimd.dma_start(sel2[:], sel_d[:])
            nc.vector.memset(eps_t[:], EPS)

            val = spool.tile([128, 8], f32)
            zc = spool.tile([128, 8], f32)
            zsq = spool.tile([128, 8], f32)
            zn16 = spool.tile([128, 16], f16)
            mu2 = spool.tile([2, 8], f32)
            std = spool.tile([2, 8], f32)
            rstd = spool.tile([2, 8], f32)
            lat = spool.tile([128, 64], f16)
            h1 = spool.tile([128, 32], f16)
            h2 = spool.tile([128, 64], f16)
            out_sb = spool.tile([16, OUT_F], f32)
            nc.vector.memset(zn16[:], 0.0)

            # --- fp16 segment stream: each slot is one [128, W] tile holding
            # two segments (one per 64-partition group), transposed. The two
            # halves go down both HWDGE queues in parallel; reduction is a
            # tensor_tensor max tree (2x fp16 DVE mode) + short final reduce.
            for s in range(N_SLOTS):
                W = Ws[s]
                h = W // 2
                ft = fpool.tile([128, W0], f16, name="ft")
                bt = bpool.tile([128, W0 // 2], f16, name="bt")
                nc.sync.dma_start(ft[:, 0:h], xs_d[s][:, 0:h])
                nc.scalar.dma_start(ft[:, h:W], xs_d[s][:, h:W])
                # fold high half onto low half, ping-ponging ft <-> bt
                nc.vector.tensor_tensor(
                    bt[:, 0:h], ft[:, 0:h], ft[:, h:W], op=mybir.AluOpType.max
                )
                cur, other, w = bt, ft, h
                while w > 512:
                    nh = w // 2
                    nc.vector.tensor_tensor(
                        other[:, 0:nh], cur[:, 0:nh], cur[:, nh:w],
                        op=mybir.AluOpType.max,
                    )
                    cur, other, w = other, cur, nh
                nc.vector.reduce_max(
                    val[:, s : s + 1], cur[:, 0:w], axis=mybir.AxisListType.X
                )

            # w3 streamed last (fp16, 12 blocks in matmul order): it fills the
            # DMA queues while the LN/MLP tail runs, landing just in time.
            w3t = [w3pool.tile([128, 4, 512], f16, name=f"w3_{b}") for b in range(12)]
            for b in range(12):
                eng = nc.sync if b % 2 == 0 else nc.scalar
                eng.dma_start(w3t[b][:], w3_d[:, b, :, :])

            # --- LayerNorm per (group, col) directly on val [128, 8] ---
            red = ps_s.tile([2, 8], f32, name="red")
            nc.tensor.matmul(red[:], sel2T[:], val[:], start=True, stop=True)
            nc.scalar.mul(mu2[:], red[:], 1.0 / FEAT)
            bc = ps_b.tile([128, 8], f32, name="bc")
            nc.tensor.matmul(bc[:], sel2[:], mu2[:], start=True, stop=True)
            nc.vector.tensor_tensor(zc[:], val[:], bc[:], op=mybir.AluOpType.subtract)
            nc.scalar.activation(zsq[:], zc[:], AF.Square)
            red2 = ps_s.tile([2, 8], f32, name="red")
            nc.tensor.matmul(red2[:], sel2T[:], zsq[:], start=True, stop=True)
            nc.scalar.activation(std[:], red2[:], AF.Sqrt, bias=eps_t[:], scale=1.0 / FEAT)
            nc.vector.reciprocal(rstd[:], std[:])
            bc2 = ps_b.tile([128, 8], f32, name="bc")
            nc.tensor.matmul(bc2[:], sel2[:], rstd[:], start=True, stop=True)
            # scatter normalized groups into disjoint columns of zn16
            nc.vector.tensor_tensor(
                zn16[0:64, 0:8], zc[0:64, :], bc2[0:64, :], op=mybir.AluOpType.mult
            )
            nc.vector.tensor_tensor(
                zn16[64:128, 8:16], zc[64:128, :], bc2[64:128, :],
                op=mybir.AluOpType.mult,
            )

            # --- proj (ln affine folded into wp/bp): lat[128m+p, s] ---
            for m in range(4):
                ps = ps_m.tile([128, 16], f32)
                nc.tensor.matmul(
                    ps[:], wp[:, 128 * m : 128 * (m + 1)], zn16[:],
                    start=True, stop=True,
                )
                nc.scalar.activation(
                    lat[:, 16 * m : 16 * (m + 1)], ps[:], AF.Identity,
                    bias=bp[:, m : m + 1],
                )

            # --- h1 = relu(latent @ w1 + b1), transposed ---
            for n in range(2):
                ps = ps_m.tile([128, 16], f32)
                for k in range(4):
                    nc.tensor.matmul(
                        ps[:],
                        w1[:, (k * 2 + n) * 128 : (k * 2 + n + 1) * 128],
                        lat[:, 16 * k : 16 * (k + 1)],
                        start=(k == 0), stop=(k == 3),
                    )
                nc.scalar.activation(
                    h1[:, 16 * n : 16 * (n + 1)], ps[:], AF.Relu,
                    bias=b1[:, n : n + 1],
                )

            # --- h2 = relu(h1 @ w2 + b2), transposed ---
            for n in range(4):
                ps = ps_m.tile([128, 16], f32)
                for k in range(2):
                    nc.tensor.matmul(
                        ps[:],
                        w2[:, (k * 4 + n) * 128 : (k * 4 + n + 1) * 128],
                        h1[:, 16 * k : 16 * (k + 1)],
                        start=(k == 0), stop=(k == 1),
                    )
                nc.scalar.activation(
                    h2[:, 16 * n : 16 * (n + 1)], ps[:], AF.Relu,
                    bias=b2[:, n : n + 1],
                )

            # --- out[s, n] = h2.T @ w3: h2 as stationary lhsT, one DMA out ---
            for b in range(12):
                ps = ps_o.tile([16, 512], f32, name="pso")
                for k in range(4):
                    nc.tensor.matmul(
                        ps[:],
                        h2[:, 16 * k : 16 * (k + 1)],
                        w3t[b][:, k, :],
                        start=(k == 0), stop=(k == 3),
                    )
                dst = out_sb[:, 512 * b : 512 * (b + 1)]
                if b % 2 == 0:
                    nc.vector.tensor_copy(dst, ps[:])
                else:
                    nc.scalar.activation(dst, ps[:], AF.Identity)
            nc.sync.dma_start(out_d[:], out_sb[:])

    nc.finalize()
    return nc


def pack_weights(ln_g, ln_b, proj_w, proj_b, w1, b1, w2, b2, w3, b3):
    c = np.ascontiguousarray
    wp = (ln_g[:, None] * proj_w).astype(np.float16)  # [64, 512]
    bpv = (ln_b.astype(np.float64) @ proj_w.astype(np.float64)).astype(np.float32) + proj_b
    return {
        "wp2": c(np.vstack([wp, wp])),
        "bp": c(bpv.reshape(4, 128).T),
        "w1p": c(w1.reshape(4, 128, 2, 128).transpose(1, 0, 2, 3).reshape(128, 1024).astype(np.float16)),
        "b1p": c(b1.reshape(2, 128).T),
        "w2p": c(w2.reshape(2, 128, 4, 128).transpose(1, 0, 2, 3).reshape(128, 1024).astype(np.float16)),
        "b2p": c(b2.reshape(4, 128).T),
        # w3n[p, b, k, c] = w3[k*128 + p, 512*b + c]
        "w3n": c(w3.reshape(4, 128, 12, 512).transpose(1, 2, 0, 3).astype(np.float16)),
        "selT": c(np.repeat(np.eye(2, dtype=np.float32), 64, axis=0)),
        "sel": c(np.repeat(np.eye(2, dtype=np.float32), 64, axis=1)),
    }


def plan_layout(batch):
    """Pair segments by sorted length, snake-assign pairs to cores.

    Returns (bounds, mapping[c][g][s] -> segment id, Ws[s] widths)."""
    bounds = np.searchsorted(batch, np.arange(B + 1))
    lens = np.diff(bounds)
    order = np.argsort(-lens, kind="stable")
    mapping = np.zeros((N_CORES, 2, N_SLOTS), np.int64)
    Ws = []
    for s in range(N_SLOTS):
        w = int(lens[order[16 * s]])  # longest segment in this rank block
        Ws.append(max(64, -(-w // 32) * 32))
        for ci in range(N_CORES):
            c = ci if s % 2 == 0 else N_CORES - 1 - ci
            r = 8 * s + ci
            mapping[c, 0, s] = order[2 * r]
            mapping[c, 1, s] = order[2 * r + 1]
    return bounds, mapping, Ws


def pack_feat_core(feat16, bounds, mapping, Ws, c):
    tiles = {}
    for s in range(N_SLOTS):
        W = Ws[s]
        arr = np.full((128, W), FMIN16, np.float16)
        for g in range(2):
            seg = mapping[c, g, s]
            a, b = bounds[seg], bounds[seg + 1]
            blk = feat16[a:b]
            L = b - a
            if L > W:  # defensive; cannot happen with planned widths
                blk = np.concatenate(
                    [blk[: W - 1], blk[W - 1 :].max(0, keepdims=True)], 0
                )
                L = W
            if L > 0:
                arr[g * 64 : (g + 1) * 64, :L] = blk.T
        tiles[f"x{s}"] = arr
    return tiles


def _prepare(inputs):
    feat = np.asarray(inputs["feat"])
    batch = np.asarray(inputs["batch"])
    wdict = pack_weights(
        *(np.asarray(inputs[k], dtype=np.float32) for k in
          ("ln_g", "ln_b", "proj_w", "proj_b", "w1", "b1", "w2", "b2", "w3", "b3"))
    )
    bounds, mapping, Ws = plan_layout(batch)
    key = tuple(Ws)
    if _CACHE.get("key") != key:
        _CACHE["nc"] = build_nc(Ws)
        _CACHE["key"] = key
    nc = _CACHE["nc"]
    feat16 = feat.astype(np.float16)
    in_maps = [
        {**pack_feat_core(feat16, bounds, mapping, Ws, c), **wdict}
        for c in range(N_CORES)
    ]
    return nc, in_maps, mapping


def _unpack(res, mapping, b3):
    out = np.empty((B, OUT_F), np.float32)
    for c in range(N_CORES):
        o = res.results[c]["out"]  # [16, OUT_F]
        for g in range(2):
            for s in range(N_SLOTS):
                out[mapping[c, g, s]] = o[g * 8 + s]
    out += np.asarray(b3, dtype=np.float32)[None, :]
    return out.reshape(B, 2048, 3)


def kernel(**inputs):
    from concourse.bass_utils import run_bass_kernel_spmd

    nc, in_maps, mapping = _prepare(inputs)
    res = run_bass_kernel_spmd(nc, in_maps, list(range(N_CORES)))
    return _unpack(res, mapping, inputs["b3"])


# revision 13
# speedup vs baseline: 1.6358x; 1.0240x over previous
import numpy as np

B = 128
FEAT = 64
LATENT = 512
OUT_F = 6144  # NUM_POINTS * 3
EPS = 1e-5
N_CORES = 8
N_SLOTS = 8  # segment pairs per core
CHUNK = 2048  # stream DMA chunk columns
ACCW = 1024  # accumulator width
FMIN16 = np.float16(-65504.0)

_CACHE = {}


def build_nc(Ws):
    from concourse import bass, bacc, tile

    mybir = bass.mybir
    f32 = mybir.dt.float32
    f16 = mybir.dt.float16
    AF = mybir.ActivationFunctionType
    MAX = mybir.AluOpType.max

    nc = bacc.Bacc("TRN2")
    xs_d = [
        nc.declare_dram_parameter(f"x{s}", [128, Ws[s]], f16, isOutput=False)
        for s in range(N_SLOTS)
    ]
    wp_d = nc.declare_dram_parameter("wp2", [128, LATENT], f16, isOutput=False)
    bp_d = nc.declare_dram_parameter("bp", [128, 4], f32, isOutput=False)
    w1_d = nc.declare_dram_parameter("w1p", [128, 1024], f16, isOutput=False)
    b1_d = nc.declare_dram_parameter("b1p", [128, 2], f32, isOutput=False)
    w2_d = nc.declare_dram_parameter("w2p", [128, 1024], f16, isOutput=False)
    b2_d = nc.declare_dram_parameter("b2p", [128, 4], f32, isOutput=False)
    w3_d = nc.declare_dram_parameter("w3n", [128, 12, 4, 512], f16, isOutput=False)
    selT_d = nc.declare_dram_parameter("selT", [128, 2], f32, isOutput=False)
    sel_d = nc.declare_dram_parameter("sel", [2, 128], f32, isOutput=False)
    out_d = nc.declare_dram_parameter("out", [16, OUT_F], f32, isOutput=True)

    with tile.TileContext(nc) as tc:
        with (
            tc.tile_pool(name="wpool", bufs=1) as wpool,
            tc.tile_pool(name="cpool", bufs=8) as cpool,
            tc.tile_pool(name="apool", bufs=2) as apool,
            tc.tile_pool(name="w3pool", bufs=1) as w3pool,
            tc.tile_pool(name="spool", bufs=1) as spool,
            tc.tile_pool(name="ps_s", bufs=1, space=bass.MemorySpace.PSUM) as ps_s,
            tc.tile_pool(name="ps_b", bufs=1, space=bass.MemorySpace.PSUM) as ps_b,
            tc.tile_pool(name="ps_m", bufs=3, space=bass.MemorySpace.PSUM) as ps_m,
            tc.tile_pool(name="ps_o", bufs=3, space=bass.MemorySpace.PSUM) as ps_o,
        ):
            wp = wpool.tile([128, LATENT], f16)
            bp = wpool.tile([128, 4], f32)
            w1 = wpool.tile([128, 1024], f16)
            b1 = wpool.tile([128, 2], f32)
            w2 = wpool.tile([128, 1024], f16)
            b2 = wpool.tile([128, 4], f32)
            # small weights on the gpsimd SWDGE queue: keeps both HW queues
            # free for feat streaming from t=0
            for t, d in (
                (wp, wp_d), (bp, bp_d), (w1, w1_d), (b1, b1_d),
                (w2, w2_d), (b2, b2_d),
            ):
                nc.gpsimd.dma_start(t[:], d[:])

            # group-selector matrices for the two-group LayerNorm
            sel2T = wpool.tile([128, 2], f32)  # sel2T[p, j] = (p // 64 == j)
            sel2 = wpool.tile([2, 128], f32)  # sel2[j, p] = (p // 64 == j)
            eps_t = wpool.tile([2, 1], f32)
            warm = wpool.tile([2, 1], f32)
            nc.gpsimd.dma_start(sel2T[:], selT_d[:])
            nc.gpsimd.dma_start(sel2[:], sel_d[:])
            nc.vector.memset(eps_t[:], EPS)
            # preload the Sqrt activation table while the stream runs
            nc.scalar.activation(warm[:], eps_t[:], AF.Sqrt)

            # stat16 cols 0:8 = segment max per slot, cols 8:16 = its square
            stat16 = spool.tile([128, 16], f32)
            zc = spool.tile([128, 8], f32)
            zn16 = spool.tile([128, 16], f16)
            bcr = spool.tile([2, 16], f32)  # cols 0:8 mu, cols 8:16 rstd
            musq = spool.tile([2, 8], f32)
            var = spool.tile([2, 8], f32)
            std = spool.tile([2, 8], f32)
            lat = spool.tile([128, 64], f16)
            h1 = spool.tile([128, 32], f16)
            h2 = spool.tile([128, 64], f16)
            out_sb = spool.tile([16, OUT_F], f32)
            nc.vector.memset(zn16[:], 0.0)

            # --- fp16 segment stream, chunked; running max into ping-pong
            # accumulators (tensor_tensor max gets the 2x fp16 DVE mode) ---
            q = 0
            for s in range(N_SLOTS):
                W = Ws[s]
                acc = [
                    apool.tile([128, ACCW], f16, name="accA"),
                    apool.tile([128, ACCW], f16, name="accB"),
                ]
                nsub = 0
                for off in range(0, W, CHUNK):
                    cw = min(CHUNK, W - off)
                    ct = cpool.tile([128, CHUNK], f16, name="ct")
                    eng = nc.sync if q == 0 else nc.scalar
                    q ^= 1
                    eng.dma_start(ct[:, 0:cw], xs_d[s][:, off : off + cw])
                    # pad the short tail chunk so every ping-pong max covers
                    # the full ACCW width (a partial write would leave stale
                    # columns in the other buffer out of the running max)
                    cw_eff = -(-cw // ACCW) * ACCW
                    if cw_eff > cw:
                        nc.vector.memset(ct[:, cw:cw_eff], -65504.0)
                    for so in range(0, cw_eff, ACCW):
                        src = ct[:, so : so + ACCW]
                        if nsub == 0:
                            nc.vector.tensor_copy(acc[0][:], src)
                        else:
                            nc.vector.tensor_tensor(
                                acc[nsub % 2][:],
                                acc[(nsub + 1) % 2][:],
                                src, op=MAX,
                            )
                        nsub += 1
                nc.vector.reduce_max(
                    stat16[:, s : s + 1],
                    acc[(nsub + 1) % 2][:, 0:ACCW],
                    axis=mybir.AxisListType.X,
                )

            # w3 streamed last (fp16, 12 blocks in matmul order): it fills the
            # DMA queues while the LN/MLP tail runs, landing just in time.
            w3t = [w3pool.tile([128, 4, 512], f16, name=f"w3_{b}") for b in range(12)]
            for b in range(12):
                eng = nc.sync if b % 2 == 0 else nc.scalar
                eng.dma_start(w3t[b][:], w3_d[:, b, :, :])

            # --- LayerNorm per (group, col): joint sum/sumsq matmul, DVE
            # small ops, single (prewarmed) Rsqrt on the scalar engine ---
            nc.vector.tensor_tensor(
                stat16[:, 8:16], stat16[:, 0:8], stat16[:, 0:8],
                op=mybir.AluOpType.mult,
            )
            red = ps_s.tile([2, 16], f32, name="red")
            nc.tensor.matmul(red[:], sel2T[:], stat16[:], start=True, stop=True)
            nc.vector.tensor_scalar_mul(bcr[:, 0:8], red[:, 0:8], 1.0 / FEAT)
            nc.vector.tensor_tensor(
                musq[:], bcr[:, 0:8], bcr[:, 0:8], op=mybir.AluOpType.mult
            )
            nc.vector.tensor_scalar_mul(var[:], red[:, 8:16], 1.0 / FEAT)
            nc.vector.tensor_tensor(
                var[:], var[:], musq[:], op=mybir.AluOpType.subtract
            )
            nc.scalar.activation(std[:], var[:], AF.Sqrt, bias=eps_t[:])
            nc.vector.reciprocal(bcr[:, 8:16], std[:])
            bc = ps_b.tile([128, 16], f32, name="bc")
            nc.tensor.matmul(bc[:], sel2[:], bcr[:], start=True, stop=True)
            nc.vector.tensor_tensor(
                zc[:], stat16[:, 0:8], bc[:, 0:8], op=mybir.AluOpType.subtract
            )
            # scatter normalized groups into disjoint columns of zn16
            nc.vector.tensor_tensor(
                zn16[0:64, 0:8], zc[0:64, :], bc[0:64, 8:16], op=mybir.AluOpType.mult
            )
            nc.vector.tensor_tensor(
                zn16[64:128, 8:16], zc[64:128, :], bc[64:128, 8:16],
                op=mybir.AluOpType.mult,
            )

            # --- proj (ln affine folded into wp/bp): lat[128m+p, s] ---
            for m in range(4):
                ps = ps_m.tile([128, 16], f32)
                nc.tensor.matmul(
                    ps[:], wp[:, 128 * m : 128 * (m + 1)], zn16[:],
                    start=True, stop=True,
                )
                nc.vector.tensor_scalar_add(
                    lat[:, 16 * m : 16 * (m + 1)], ps[:], bp[:, m : m + 1]
                )

            # --- h1 = relu(latent @ w1 + b1), transposed ---
            for n in range(2):
                ps = ps_m.tile([128, 16], f32)
                for k in range(4):
                    nc.tensor.matmul(
                        ps[:],
                        w1[:, (k * 2 + n) * 128 : (k * 2 + n + 1) * 128],
                        lat[:, 16 * k : 16 * (k + 1)],
                        start=(k == 0), stop=(k == 3),
                    )
                nc.vector.tensor_scalar(
                    h1[:, 16 * n : 16 * (n + 1)], ps[:],
                    b1[:, n : n + 1], 0.0,
                    op0=mybir.AluOpType.add, op1=MAX,
                )

            # --- h2 = relu(h1 @ w2 + b2), transposed ---
            for n in range(4):
                ps = ps_m.tile([128, 16], f32)
                for k in range(2):
                    nc.tensor.matmul(
                        ps[:],
                        w2[:, (k * 4 + n) * 128 : (k * 4 + n + 1) * 128],
                        h1[:, 16 * k : 16 * (k + 1)],
                        start=(k == 0), stop=(k == 1),
                    )
                nc.vector.tensor_scalar(
                    h2[:, 16 * n : 16 * (n + 1)], ps[:],
                    b2[:, n : n + 1], 0.0,
                    op0=mybir.AluOpType.add, op1=MAX,
                )

            # --- out[s, n] = h2.T @ w3: copies rotate V/Pool/S engines,
            # output DMA in 3 chunks so it overlaps the last copies ---
            for b in range(12):
                ps = ps_o.tile([16, 512], f32, name="pso")
                for k in range(4):
                    nc.tensor.matmul(
                        ps[:],
                        h2[:, 16 * k : 16 * (k + 1)],
                        w3t[b][:, k, :],
                        start=(k == 0), stop=(k == 3),
                    )
                dst = out_sb[:, 512 * b : 512 * (b + 1)]
                if b % 2 == 0:
                    nc.vector.tensor_copy(dst, ps[:])
                else:
                    nc.scalar.activation(dst, ps[:], AF.Identity)
                if b % 4 == 3:
                    g = b // 4
                    nc.sync.dma_start(
                        out_d[:, 2048 * g : 2048 * (g + 1)],
                        out_sb[:, 2048 * g : 2048 * (g + 1)],
                    )

    nc.finalize()
    return nc


def pack_weights(ln_g, ln_b, proj_w, proj_b, w1, b1, w2, b2, w3, b3):
    c = np.ascontiguousarray
    wp = (ln_g[:, None] * proj_w).astype(np.float16)  # [64, 512]
    bpv = (ln_b.astype(np.float64) @ proj_w.astype(np.float64)).astype(np.float32) + proj_b
    return {
        "wp2": c(np.vstack([wp, wp])),
        "bp": c(bpv.reshape(4, 128).T),
        "w1p": c(w1.reshape(4, 128, 2, 128).transpose(1, 0, 2, 3).reshape(128, 1024).astype(np.float16)),
        "b1p": c(b1.reshape(2, 128).T),
        "w2p": c(w2.reshape(2, 128, 4, 128).transpose(1, 0, 2, 3).reshape(128, 1024).astype(np.float16)),
        "b2p": c(b2.reshape(4, 128).T),
        # w3n[p, b, k, c] = w3[k*128 + p, 512*b + c]
        "w3n": c(w3.reshape(4, 128, 12, 512).transpose(1, 2, 0, 3).astype(np.float16)),
        "selT": c(np.repeat(np.eye(2, dtype=np.float32), 64, axis=0)),
        "sel": c(np.repeat(np.eye(2, dtype=np.float32), 64, axis=1)),
    }


def plan_layout(batch):
    """Pair segments by sorted length, snake-assign pairs to cores.

    Returns (bounds, mapping[c][g][s] -> segment id, Ws[s] widths)."""
    bounds = np.searchsorted(batch, np.arange(B + 1))
    lens = np.diff(bounds)
    order = np.argsort(-lens, kind="stable")
    mapping = np.zeros((N_CORES, 2, N_SLOTS), np.int64)
    Ws = []
    for s in range(N_SLOTS):
        w = int(lens[order[16 * s]])  # longest segment in this rank block
        Ws.append(max(64, -(-w // 32) * 32))
        for ci in range(N_CORES):
            c = ci if s % 2 == 0 else N_CORES - 1 - ci
            r = 8 * s + ci
            mapping[c, 0, s] = order[2 * r]
            mapping[c, 1, s] = order[2 * r + 1]
    return bounds, mapping, Ws


def pack_feat_core(feat16, bounds, mapping, Ws, c):
    tiles = {}
    for s in range(N_SLOTS):
        W = Ws[s]
        arr = np.full((128, W), FMIN16, np.float16)
        for g in range(2):
            seg = mapping[c, g, s]
            a, b = bounds[seg], bounds[seg + 1]
            blk = feat16[a:b]
            L = b - a
            if L > W:  # defensive; cannot happen with planned widths
                blk = np.concatenate(
                    [blk[: W - 1], blk[W - 1 :].max(0, keepdims=True)], 0
                )
                L = W
            if L > 0:
                arr[g * 64 : (g + 1) * 64, :L] = blk.T
        tiles[f"x{s}"] = arr
    return tiles


def _prepare(inputs):
    feat = np.asarray(inputs["feat"])
    batch = np.asarray(inputs["batch"])
    wdict = pack_weights(
        *(np.asarray(inputs[k], dtype=np.float32) for k in
          ("ln_g", "ln_b", "proj_w", "proj_b", "w1", "b1", "w2", "b2", "w3", "b3"))
    )
    bounds, mapping, Ws = plan_layout(batch)
    key = tuple(Ws)
    if _CACHE.get("key") != key:
        _CACHE["nc"] = build_nc(Ws)
        _CACHE["key"] = key
    nc = _CACHE["nc"]
    feat16 = feat.astype(np.float16)
    in_maps = [
        {**pack_feat_core(feat16, bounds, mapping, Ws, c), **wdict}
        for c in range(N_CORES)
    ]
    return nc, in_maps, mapping


def _unpack(res, mapping, b3):
    out = np.empty((B, OUT_F), np.float32)
    for c in range(N_CORES):
        o = res.results[c]["out"]  # [16, OUT_F]
        for g in range(2):
            for s in range(N_SLOTS):
                out[mapping[c, g, s]] = o[g * 8 + s]
    out += np.asarray(b3, dtype=np.float32)[None, :]
    return out.reshape(B, 2048, 3)


def kernel(**inputs):
    from concourse.bass_utils import run_bass_kernel_spmd

    nc, in_maps, mapping = _prepare(inputs)
    res = run_bass_kernel_spmd(nc, in_maps, list(range(N_CORES)))
    return _unpack(res, mapping, inputs["b3"])
